# revision 15
# baseline (speedup 1.0000x reference)
"""ContrastiveLoss Trainium2 kernel (8 NeuronCores, SPMD row-sharded,
symmetry-exploiting).

Math (reference):
    f = features / ||features||_row            (L2 normalize)
    s_ij = (f_i . f_j) / T,  T = 0.1
    Z_i = sum_{j != i} exp(s_ij)
    per_row_i = (npos_i * ln(Z_i) - sum_{j in pos, j != i} s_ij) / (npos_i + eps)
    loss = mean(per_row)

Device computes ONLY the O(N^2) part: partial sums of exp(10 * f_i . f_j).
Each unordered (i, j) pair is computed exactly once across the 8 cores
(N^2/2 total work): core k owns rows [k*1024, (k+1)*1024) and computes
local columns [0, 4608) of its rotated column space:
  [0, 1024)     own (diagonal) block, self-terms masked
  [1024, 4096)  the next 3 row-blocks (d = 1, 2, 3)
  [4096, 4608)  a checkerboard half of the d = 4 block: row-half rb0-3 gets
                one 512-column half, rb4-7 the other; cores 4-7 receive the
                two halves host-swapped so the 4 quarters of each d=4 block
                pair are covered exactly once globally.
Row sums (ACT accum) give Z contributions for the core's own rows; column
sums (DoubleRow ones-matmuls over fp8e5 exp tiles) give the contributions
to the partner rows, which the host scatters back. All O(N*D) math
(normalize, transpose, class sums, final assembly) runs on the host in f32.
"""

import numpy as np
import ml_dtypes

TEMP_INV = 10.0  # 1/temperature
EPS = 1e-8
N, D, NCORES = 8192, 512, 8
RPC = N // NCORES        # 1024 rows per core
RT = RPC // 128          # 8 row tiles (128 rows) per core
KC = D // 128            # 4 contraction chunks of 128
GW = 1536                # psum group width (3 banks)
NB = 3                   # bands (psum groups) per row tile
LC = 5120                # local columns resident in SBUF
CC = 4096                # columns with column-sum output (local 1024..5120)
WARMUP_MMS = 40          # dummy matmuls to warm the PE HAM clock gate

# per-band matmul column offsets in SBUF local-column space; band 2's third
# tile is the d4 half: offset 4096 + 512*(rb//4)
BAND_COLS = [(0, 512, 1024), (1536, 2048, 2560), (3072, 3584, None)]
BAND_ORDER = [0, 1, 2]  # band 0 first: it holds the lhsT columns
# colsum chunks per band: (psum offset, colacc offset or "d4")
BAND_CS = [((1024, 0),),
           ((0, 512), (512, 1024), (1024, 1536)),
           ((0, 2048), (512, 2560), (1024, "d4"))]

_prog_cache = None


def _build_program():
    import concourse.bacc as bacc
    import concourse.tile as tile
    from concourse import mybir

    f32, bf16 = mybir.dt.float32, mybir.dt.bfloat16
    fp8, fp8e5 = mybir.dt.float8e4, mybir.dt.float8e5
    A = mybir.ActivationFunctionType
    Alu = mybir.AluOpType
    X = mybir.AxisListType.X
    DR = mybir.MatmulPerfMode.DoubleRow

    nc = bacc.Bacc("TRN2", target_bir_lowering=False, debug=False,
                   num_devices=NCORES)

    # band-major packed transposed features, [128, KC*LC] fp8e4
    fT8d = nc.dram_tensor("fT8", [128, KC * LC], fp8, kind="ExternalInput")
    masksd = nc.dram_tensor("masks", [128, 256], fp8, kind="ExternalInput")
    onesd = nc.dram_tensor("ones8", [128, 2, 16], fp8e5, kind="ExternalInput")
    zoutd = nc.dram_tensor("zout", [128, RT], f32, kind="ExternalOutput")
    coutd = nc.dram_tensor("cout", [1, CC], f32, kind="ExternalOutput")

    from contextlib import ExitStack

    with tile.TileContext(nc) as tc, ExitStack() as ctx:
        singles = ctx.enter_context(tc.tile_pool(name="singles", bufs=1))
        esc8p = ctx.enter_context(tc.tile_pool(name="esc8p", bufs=3))

        fb0 = singles.tile([128, KC, 1536], fp8, tag="fb0", name="fb0")
        fb1 = singles.tile([128, KC, 1536], fp8, tag="fb1", name="fb1")
        fb2 = singles.tile([128, KC, 2048], fp8, tag="fb2", name="fb2")
        fbands = [fb0, fb1, fb2]
        masks = singles.tile([128, 256], fp8, tag="masks")
        ones8 = singles.tile([128, 2, 16], fp8e5, tag="ones8")
        warm = singles.tile([128, 256], fp8, tag="warm")
        dumm = singles.tile([128, 256], bf16, tag="dumm")
        zacs = singles.tile([128, RT, NB], f32, tag="zacs")
        ZE = singles.tile([128, RT], f32, tag="ZE")
        colacc = singles.tile([1, CC], f32, tag="colacc")

        nc.vector.memset(warm, 0.0)
        nc.vector.memset(colacc, 0.0)

        # DMA: per-band tiles; each partition's slice is contiguous KC*w
        band_widths = {0: 1536, 1: 1536, 2: 2048}
        off = 0
        for b in BAND_ORDER:
            w = band_widths[b]
            nc.sync.dma_start(
                out=fbands[b],
                in_=fT8d.ap()[:, off:off + KC * w].rearrange(
                    "p (k w) -> p k w", k=KC))
            off += KC * w
        nc.gpsimd.dma_start(out=masks, in_=masksd.ap())
        nc.gpsimd.dma_start(out=ones8, in_=onesd.ap())

        I_sb = masks[:, 0:128]
        negI_sb = masks[:, 128:256]

        with tc.tile_pool(name="mps", bufs=2, space="PSUM") as mpp, \
                tc.tile_pool(name="cps", bufs=2, space="PSUM") as cpp:
            # warm the PE + trigger the ACT exp table load while DMA streams
            wps = mpp.tile([128, GW], f32, tag="ps")
            for w in range(WARMUP_MMS):
                nc.tensor.matmul(wps[:, :128], lhsT=warm[:, :128],
                                 rhs=warm[:, 128:], start=True, stop=True)
            nc.scalar.activation(out=dumm, in_=warm, func=A.Exp,
                                 scale=TEMP_INV)

            esc_pair = [None]
            pend = []  # deferred colsums: (ready_gidx, band, pair, esc tile)
            cs_q = []  # colsum closures, popped one per main matmul slot

            def one_colsum(ps_off, ca_off, esc8):
                cs = cpp.tile([1, 512], f32, tag="cs")
                nc.tensor.matmul(
                    cs, lhsT=ones8[:, :, 0:1],
                    rhs=esc8[:, :, ps_off:ps_off + 512],
                    perf_mode=DR, start=True, stop=True,
                    skip_group_check=True)
                nc.vector.tensor_tensor(
                    out=colacc[:, ca_off:ca_off + 512],
                    in0=colacc[:, ca_off:ca_off + 512],
                    in1=cs, op=Alu.add)

            def queue_colsums(band, pair, esc8):
                for ps_off, ca_off in BAND_CS[band]:
                    if ca_off == "d4":
                        ca_off = 3072 + 512 * (pair // 2)
                    cs_q.append((ps_off, ca_off, esc8))

            gidx = 0
            for band in BAND_ORDER:
                for rb in range(RT):
                    ps = mpp.tile([128, GW], f32, tag="ps")
                    if rb % 2 == 0:
                        esc_pair[0] = esc8p.tile([128, 2, GW], fp8e5,
                                                 tag="esc8", name="esc8")
                    esc8 = esc_pair[0]
                    fb = fbands[band]
                    for kc2 in range(2):
                        for ct in range(3):
                            c0 = 512 * ct
                            if band == 2 and ct == 2:
                                c0 = 1024 + 512 * (rb // 4)
                            diag_here = (band == 0 and ct == rb // 4)
                            nc.tensor.matmul(
                                ps[:, 512 * ct:512 * ct + 512],
                                lhsT=fb0[:, 2 * kc2:2 * kc2 + 2,
                                         128 * rb:128 * rb + 128],
                                rhs=fb[:, 2 * kc2:2 * kc2 + 2, c0:c0 + 512],
                                perf_mode=DR,
                                start=(kc2 == 0),
                                stop=(kc2 == 1 and not diag_here))
                            if cs_q:
                                one_colsum(*cs_q.pop(0))
                    if band == 0:
                        # mask own diagonal: adds -96 at ps[p, rb*128+p]
                        nc.tensor.matmul(
                            ps[:, 128 * rb:128 * rb + 128],
                            lhsT=I_sb, rhs=negI_sb,
                            start=False, stop=True, skip_group_check=True)
                    # deferred colsums ride the PE queue behind these matmuls
                    while pend and pend[0][0] <= gidx:
                        _, b_, p_, e_ = pend.pop(0)
                        queue_colsums(b_, p_, e_)
                    nc.scalar.activation(out=esc8[:, rb % 2], in_=ps,
                                         func=A.Exp, scale=TEMP_INV,
                                         accum_out=zacs[:, rb, band:band + 1])
                    if rb % 2 == 1:
                        pend.append((gidx + 2, band, rb // 2, esc8))
                    if band == BAND_ORDER[-1]:
                        nc.vector.tensor_reduce(out=ZE[:, rb:rb + 1],
                                                in_=zacs[:, rb], axis=X,
                                                op=Alu.add)
                    gidx += 1
            while pend:
                _, b_, p_, e_ = pend.pop(0)
                queue_colsums(b_, p_, e_)
            while cs_q:
                one_colsum(*cs_q.pop(0))

        nc.sync.dma_start(out=zoutd.ap(), in_=ZE)
        nc.sync.dma_start(out=coutd.ap(), in_=colacc)

    nc.compile()
    return nc


def _get_program():
    global _prog_cache
    if _prog_cache is None:
        _prog_cache = _build_program()
    return _prog_cache


def _prep_inputs(features, labels):
    f8t = ml_dtypes.float8_e4m3
    f = np.asarray(features, dtype=np.float32)
    lab = np.asarray(labels).astype(np.int64)

    norm = np.maximum(np.sqrt((f * f).sum(axis=1, keepdims=True)), 1e-12)
    fn = f / norm                                   # [N, D] f32, unit rows
    f8 = fn.astype(f8t)                             # device values

    # fT8_full[p, kc, j] = f8[j, kc*128 + p]
    fT8_full = np.ascontiguousarray(
        f8.T.reshape(KC, 128, N).transpose(1, 0, 2))
    fT8_dbl = np.concatenate([fT8_full, fT8_full], axis=2)

    masks = np.zeros((128, 256), np.float32)
    masks[:, 0:128] = np.eye(128)
    masks[:, 128:256] = np.eye(128) * -96.0
    masks = masks.astype(f8t)
    ones8 = np.ones((128, 2, 16), np.float32).astype(ml_dtypes.float8_e5m2)

    in_maps = []
    for k in range(NCORES):
        loc = fT8_dbl[:, :, k * RPC:k * RPC + LC].copy()
        if k >= 4:
            # swap the two 512-col halves of the d=4 block
            loc[:, :, 4096:5120] = np.concatenate(
                [loc[:, :, 4608:5120], loc[:, :, 4096:4608]], axis=2)
        packed = np.concatenate(
            [loc[:, :, c0:c0 + w].reshape(128, -1)
             for c0, w in ((0, 1536), (1536, 1536),
                           (3072, 2048))], axis=1)
        in_maps.append({
            "fT8": np.ascontiguousarray(packed),
            "masks": masks,
            "ones8": ones8,
        })

    # host-side O(N*D) terms, f32 like the reference
    oh = np.stack([lab == 0, lab == 1], axis=1).astype(np.float32)
    counts = oh.sum(axis=0)
    npos = (counts[lab] - 1).astype(np.float32)     # positives excl. self
    g = fn.T @ oh                                   # [D, 2] class sums
    rddot = (fn @ g)[np.arange(N), lab]             # f_i . g_{lab_i}
    sii = (fn * fn).sum(axis=1)                     # ~1.0
    possum = TEMP_INV * (rddot - sii)               # sum_{j in pos, j!=i} s_ij
    return in_maps, npos, possum


def _colsum_global_cols(k):
    """Global column index for each of core k's CC colsum outputs."""
    t = np.arange(CC)
    g = (k * RPC + 1024 + t) % N
    if k >= 4:
        a = ((k + 4) % 8) * RPC
        d4 = t >= 3072
        tl = t[(t >= 3072) & (t < 3584)]
        tr = t[t >= 3584]
        g = g.copy()
        g[tl] = a + 512 + (tl - 3072)
        g[tr] = a + (tr - 3584)
    return g


def _run(inputs, trace=False, trace_kwargs=None):
    from concourse.bass_utils import run_bass_kernel_spmd

    nc = _get_program()
    in_maps, npos, possum = _prep_inputs(inputs["features"], inputs["labels"])
    res = run_bass_kernel_spmd(nc, in_maps, core_ids=list(range(NCORES)),
                               trace=trace, **(trace_kwargs or {}))
    Z = np.zeros((N,), np.float64)
    for k in range(NCORES):
        # zout[p, rb] is the rowsum of global row k*RPC + rb*128 + p
        Z[k * RPC:(k + 1) * RPC] = res.results[k]["zout"].T.reshape(RPC)
    for k in range(NCORES):
        cs = res.results[k]["cout"].reshape(CC).astype(np.float64)
        np.add.at(Z, _colsum_global_cols(k), cs)
    lnZ = np.log(Z)
    per_row = (npos * lnZ - possum) / (npos + EPS)
    loss = np.float32(per_row.mean())
    return loss, res


def kernel(**inputs) -> np.ndarray:
    loss, _ = _run(inputs, trace=False)
    return np.asarray(loss, dtype=np.float32)


# revision 16
# speedup vs baseline: 1.0046x; 1.0046x over previous
"""ContrastiveLoss Trainium2 kernel (8 NeuronCores, SPMD row-sharded,
symmetry-exploiting).

Math (reference):
    f = features / ||features||_row            (L2 normalize)
    s_ij = (f_i . f_j) / T,  T = 0.1
    Z_i = sum_{j != i} exp(s_ij)
    per_row_i = (npos_i * ln(Z_i) - sum_{j in pos, j != i} s_ij) / (npos_i + eps)
    loss = mean(per_row)

Device computes ONLY the O(N^2) part: partial sums of exp(10 * f_i . f_j).
Each unordered (i, j) pair is computed exactly once across the 8 cores
(N^2/2 total work): core k owns rows [k*1024, (k+1)*1024) and computes
local columns [0, 4608) of its rotated column space:
  [0, 1024)     own (diagonal) block, self-terms masked
  [1024, 4096)  the next 3 row-blocks (d = 1, 2, 3)
  [4096, 4608)  a checkerboard half of the d = 4 block: row-half rb0-3 gets
                one 512-column half, rb4-7 the other; cores 4-7 receive the
                two halves host-swapped so the 4 quarters of each d=4 block
                pair are covered exactly once globally.
Row sums (ACT accum) give Z contributions for the core's own rows; column
sums (DoubleRow ones-matmuls over fp8e5 exp tiles) give the contributions
to the partner rows, which the host scatters back. All O(N*D) math
(normalize, transpose, class sums, final assembly) runs on the host in f32.
"""

import numpy as np
import ml_dtypes

TEMP_INV = 10.0  # 1/temperature
EPS = 1e-8
N, D, NCORES = 8192, 512, 8
RPC = N // NCORES        # 1024 rows per core
RT = RPC // 128          # 8 row tiles (128 rows) per core
KC = D // 128            # 4 contraction chunks of 128
GW = 1536                # psum group width (3 banks)
NB = 3                   # bands (psum groups) per row tile
LC = 5120                # local columns resident in SBUF
CC = 4096                # columns with column-sum output (local 1024..5120)
WARMUP_MMS = 32          # dummy matmuls to warm the PE HAM clock gate

# per-band matmul column offsets in SBUF local-column space; band 2's third
# tile is the d4 half: offset 4096 + 512*(rb//4)
BAND_COLS = [(0, 512, 1024), (1536, 2048, 2560), (3072, 3584, None)]
BAND_ORDER = [0, 1, 2]  # band 0 first: it holds the lhsT columns
# colsum chunks per band: (psum offset, colacc offset or "d4")
BAND_CS = [((1024, 0),),
           ((0, 512), (512, 1024), (1024, 1536)),
           ((0, 2048), (512, 2560), (1024, "d4"))]

_prog_cache = None


def _build_program():
    import concourse.bacc as bacc
    import concourse.tile as tile
    from concourse import mybir

    f32, bf16 = mybir.dt.float32, mybir.dt.bfloat16
    fp8, fp8e5 = mybir.dt.float8e4, mybir.dt.float8e5
    A = mybir.ActivationFunctionType
    Alu = mybir.AluOpType
    X = mybir.AxisListType.X
    DR = mybir.MatmulPerfMode.DoubleRow

    nc = bacc.Bacc("TRN2", target_bir_lowering=False, debug=False,
                   num_devices=NCORES)

    # band-major packed transposed features, [128, KC*LC] fp8e4
    fT8d = nc.dram_tensor("fT8", [128, KC * LC], fp8, kind="ExternalInput")
    masksd = nc.dram_tensor("masks", [128, 256], fp8, kind="ExternalInput")
    onesd = nc.dram_tensor("ones8", [128, 2, 16], fp8e5, kind="ExternalInput")
    zoutd = nc.dram_tensor("zout", [128, RT], f32, kind="ExternalOutput")
    coutd = nc.dram_tensor("cout", [1, CC], f32, kind="ExternalOutput")

    from contextlib import ExitStack

    with tile.TileContext(nc) as tc, ExitStack() as ctx:
        singles = ctx.enter_context(tc.tile_pool(name="singles", bufs=1))
        esc8p = ctx.enter_context(tc.tile_pool(name="esc8p", bufs=3))

        fb0 = singles.tile([128, KC, 1536], fp8, tag="fb0", name="fb0")
        fb1 = singles.tile([128, KC, 1536], fp8, tag="fb1", name="fb1")
        fb2 = singles.tile([128, KC, 2048], fp8, tag="fb2", name="fb2")
        fbands = [fb0, fb1, fb2]
        masks = singles.tile([128, 256], fp8, tag="masks")
        ones8 = singles.tile([128, 2, 16], fp8e5, tag="ones8")
        warm = singles.tile([128, 256], fp8, tag="warm")
        dumm = singles.tile([128, 256], bf16, tag="dumm")
        zacs = singles.tile([128, RT, NB], f32, tag="zacs")
        ZE = singles.tile([128, RT], f32, tag="ZE")
        colacc = singles.tile([1, CC], f32, tag="colacc")

        nc.vector.memset(warm, 0.0)
        nc.vector.memset(colacc, 0.0)

        # DMA: per-band tiles; each partition's slice is contiguous KC*w
        band_widths = {0: 1536, 1: 1536, 2: 2048}
        off = 0
        for b in BAND_ORDER:
            w = band_widths[b]
            nc.sync.dma_start(
                out=fbands[b],
                in_=fT8d.ap()[:, off:off + KC * w].rearrange(
                    "p (k w) -> p k w", k=KC))
            off += KC * w
        nc.gpsimd.dma_start(out=masks, in_=masksd.ap())
        nc.gpsimd.dma_start(out=ones8, in_=onesd.ap())

        I_sb = masks[:, 0:128]
        negI_sb = masks[:, 128:256]

        with tc.tile_pool(name="mps", bufs=2, space="PSUM") as mpp, \
                tc.tile_pool(name="cps", bufs=2, space="PSUM") as cpp:
            # warm the PE + trigger the ACT exp table load while DMA streams
            wps = mpp.tile([128, GW], f32, tag="ps")
            for w in range(WARMUP_MMS):
                nc.tensor.matmul(wps[:, :128], lhsT=warm[:, :128],
                                 rhs=warm[:, 128:], start=True, stop=True)
            nc.scalar.activation(out=dumm, in_=warm, func=A.Exp,
                                 scale=TEMP_INV)

            esc_pair = [None]
            pend = []  # deferred colsums: (ready_gidx, band, pair, esc tile)
            cs_q = []  # colsum closures, popped one per main matmul slot

            def one_colsum(ps_off, ca_off, esc8):
                cs = cpp.tile([1, 512], f32, tag="cs")
                nc.tensor.matmul(
                    cs, lhsT=ones8[:, :, 0:1],
                    rhs=esc8[:, :, ps_off:ps_off + 512],
                    perf_mode=DR, start=True, stop=True,
                    skip_group_check=True)
                nc.vector.tensor_tensor(
                    out=colacc[:, ca_off:ca_off + 512],
                    in0=colacc[:, ca_off:ca_off + 512],
                    in1=cs, op=Alu.add)

            def queue_colsums(band, pair, esc8):
                for ps_off, ca_off in BAND_CS[band]:
                    if ca_off == "d4":
                        ca_off = 3072 + 512 * (pair // 2)
                    cs_q.append((ps_off, ca_off, esc8))

            gidx = 0
            for band in BAND_ORDER:
                for rb in range(RT):
                    ps = mpp.tile([128, GW], f32, tag="ps")
                    if rb % 2 == 0:
                        esc_pair[0] = esc8p.tile([128, 2, GW], fp8e5,
                                                 tag="esc8", name="esc8")
                    esc8 = esc_pair[0]
                    fb = fbands[band]
                    for kc2 in range(2):
                        for ct in range(3):
                            c0 = 512 * ct
                            if band == 2 and ct == 2:
                                c0 = 1024 + 512 * (rb // 4)
                            diag_here = (band == 0 and ct == rb // 4)
                            nc.tensor.matmul(
                                ps[:, 512 * ct:512 * ct + 512],
                                lhsT=fb0[:, 2 * kc2:2 * kc2 + 2,
                                         128 * rb:128 * rb + 128],
                                rhs=fb[:, 2 * kc2:2 * kc2 + 2, c0:c0 + 512],
                                perf_mode=DR,
                                start=(kc2 == 0),
                                stop=(kc2 == 1 and not diag_here))
                            if cs_q:
                                one_colsum(*cs_q.pop(0))
                    if band == 0:
                        # mask own diagonal: adds -96 at ps[p, rb*128+p]
                        nc.tensor.matmul(
                            ps[:, 128 * rb:128 * rb + 128],
                            lhsT=I_sb, rhs=negI_sb,
                            start=False, stop=True, skip_group_check=True)
                    # deferred colsums ride the PE queue behind these matmuls
                    while pend and pend[0][0] <= gidx:
                        _, b_, p_, e_ = pend.pop(0)
                        queue_colsums(b_, p_, e_)
                    nc.scalar.activation(out=esc8[:, rb % 2], in_=ps,
                                         func=A.Exp, scale=TEMP_INV,
                                         accum_out=zacs[:, rb, band:band + 1])
                    if rb % 2 == 1:
                        pend.append((gidx + 2, band, rb // 2, esc8))
                    if band == BAND_ORDER[-1]:
                        nc.vector.tensor_reduce(out=ZE[:, rb:rb + 1],
                                                in_=zacs[:, rb], axis=X,
                                                op=Alu.add)
                    gidx += 1
            while pend:
                _, b_, p_, e_ = pend.pop(0)
                queue_colsums(b_, p_, e_)
            while cs_q:
                one_colsum(*cs_q.pop(0))

        nc.sync.dma_start(out=zoutd.ap(), in_=ZE)
        nc.sync.dma_start(out=coutd.ap(), in_=colacc)

    nc.compile()
    return nc


def _get_program():
    global _prog_cache
    if _prog_cache is None:
        _prog_cache = _build_program()
    return _prog_cache


def _prep_inputs(features, labels):
    f8t = ml_dtypes.float8_e4m3
    f = np.asarray(features, dtype=np.float32)
    lab = np.asarray(labels).astype(np.int64)

    norm = np.maximum(np.sqrt((f * f).sum(axis=1, keepdims=True)), 1e-12)
    fn = f / norm                                   # [N, D] f32, unit rows
    f8 = fn.astype(f8t)                             # device values

    # fT8_full[p, kc, j] = f8[j, kc*128 + p]
    fT8_full = np.ascontiguousarray(
        f8.T.reshape(KC, 128, N).transpose(1, 0, 2))
    fT8_dbl = np.concatenate([fT8_full, fT8_full], axis=2)

    masks = np.zeros((128, 256), np.float32)
    masks[:, 0:128] = np.eye(128)
    masks[:, 128:256] = np.eye(128) * -96.0
    masks = masks.astype(f8t)
    ones8 = np.ones((128, 2, 16), np.float32).astype(ml_dtypes.float8_e5m2)

    in_maps = []
    for k in range(NCORES):
        loc = fT8_dbl[:, :, k * RPC:k * RPC + LC].copy()
        if k >= 4:
            # swap the two 512-col halves of the d=4 block
            loc[:, :, 4096:5120] = np.concatenate(
                [loc[:, :, 4608:5120], loc[:, :, 4096:4608]], axis=2)
        packed = np.concatenate(
            [loc[:, :, c0:c0 + w].reshape(128, -1)
             for c0, w in ((0, 1536), (1536, 1536),
                           (3072, 2048))], axis=1)
        in_maps.append({
            "fT8": np.ascontiguousarray(packed),
            "masks": masks,
            "ones8": ones8,
        })

    # host-side O(N*D) terms, f32 like the reference
    oh = np.stack([lab == 0, lab == 1], axis=1).astype(np.float32)
    counts = oh.sum(axis=0)
    npos = (counts[lab] - 1).astype(np.float32)     # positives excl. self
    g = fn.T @ oh                                   # [D, 2] class sums
    rddot = (fn @ g)[np.arange(N), lab]             # f_i . g_{lab_i}
    sii = (fn * fn).sum(axis=1)                     # ~1.0
    possum = TEMP_INV * (rddot - sii)               # sum_{j in pos, j!=i} s_ij
    return in_maps, npos, possum


def _colsum_global_cols(k):
    """Global column index for each of core k's CC colsum outputs."""
    t = np.arange(CC)
    g = (k * RPC + 1024 + t) % N
    if k >= 4:
        a = ((k + 4) % 8) * RPC
        d4 = t >= 3072
        tl = t[(t >= 3072) & (t < 3584)]
        tr = t[t >= 3584]
        g = g.copy()
        g[tl] = a + 512 + (tl - 3072)
        g[tr] = a + (tr - 3584)
    return g


def _run(inputs, trace=False, trace_kwargs=None):
    from concourse.bass_utils import run_bass_kernel_spmd

    nc = _get_program()
    in_maps, npos, possum = _prep_inputs(inputs["features"], inputs["labels"])
    res = run_bass_kernel_spmd(nc, in_maps, core_ids=list(range(NCORES)),
                               trace=trace, **(trace_kwargs or {}))
    Z = np.zeros((N,), np.float64)
    for k in range(NCORES):
        # zout[p, rb] is the rowsum of global row k*RPC + rb*128 + p
        Z[k * RPC:(k + 1) * RPC] = res.results[k]["zout"].T.reshape(RPC)
    for k in range(NCORES):
        cs = res.results[k]["cout"].reshape(CC).astype(np.float64)
        np.add.at(Z, _colsum_global_cols(k), cs)
    lnZ = np.log(Z)
    per_row = (npos * lnZ - possum) / (npos + EPS)
    loss = np.float32(per_row.mean())
    return loss, res


def kernel(**inputs) -> np.ndarray:
    loss, _ = _run(inputs, trace=False)
    return np.asarray(loss, dtype=np.float32)
